# revision 6
# baseline (speedup 1.0000x reference)
"""BinarizeConv2dSDP kernel for Trainium2 (8 NeuronCores, data-parallel over batch).

out = conv2d(sign(x), sign(M + sum_k rv[k] * Z[k]), stride 1, pad 1) * Alpha

Key simplification: the reference normalizes (M, Z) by rsqrt(M^2 + sum Z^2 / SCALE)
before forming w = rv@Z + M, but that factor is strictly positive and applied
multiplicatively to the whole expression, so sign(w) is unaffected.  The binary
weights are just sign(M + sum_k rv[k] Z[k]).

Strategy per core (8 images each):
  - every bulk HBM load is split into two halves issued on the SP (sync) and
    ACT (scalar) HWDGE rings concurrently: two issue engines keep the 16 DMA
    queues fed (~1.4x the effective startup bandwidth of a single FIFO ring),
    and the weight stream (M, Z) goes out ahead of most of image 0 so the
    weight-gen chain finishes early.
  - weight gen on DVE per ic-half: w_h = (rv0*z0_h + M_h), then += rv_k z_k_h,
    sign -> bf16 per half on ACT as soon as that half's chain is done.
    M is folded into the first scalar_tensor_tensor so no separate add.
  - 9 PE transposes (against an anti-diagonal permutation) produce the
    column-reversed, pair-interleaved fp8e4 weight layout that
    DoubleRowSwInterleave expects (plus one all-zero tap so 9 taps = 5 pairs).
  - conv: 5 fp8 DoubleRowSwInterleave matmuls per 8-row chunk (2 taps per
    matmul, K=256 effective contraction), accumulated in PSUM over a 58-wide
    zero-padded sign(x) image; the free dim spans whole padded rows (464) so
    the moving AP stays 3D, leaving 2 garbage columns per row that the
    eviction skips.
  - Alpha scaling on DVE during PSUM->SBUF eviction, emitting float16
    (exact conv integers * f32 alpha rounded to fp16: rel err ~5e-4, well
    inside the 2e-2 gate); the host upcasts to f32 after the gather.  fp16
    halves the output HBM traffic and the end-of-kernel drain tail.
  - steady-state output DMAs ride the SWDGE (GpSimd) ring so they never
    head-of-line block input loads on the HWDGE rings (the last two images'
    outputs switch to the then-idle SP ring).
"""

import numpy as np
from contextlib import ExitStack

import concourse.bass as bass
import concourse.mybir as mybir
import concourse.tile as tile
from concourse.bacc import Bacc
from concourse.bass_utils import run_bass_kernel_spmd

N_CORES = 8
B, C, H, W = 64, 128, 56, 56
BPC = B // N_CORES  # images per core
KS, K = 3, 5
PH, PW = H + 2, W + 2  # zero-padded image
CHUNK_ROWS = 8
N_CHUNKS = H // CHUNK_ROWS
FREE = CHUNK_ROWS * W  # valid output elements per chunk (448)
FREE_R = CHUNK_ROWS * PW  # matmul free dim incl. garbage cols (464 <= 512)
F32 = mybir.dt.float32
F16 = mybir.dt.float16
BF16 = mybir.dt.bfloat16
F8 = mybir.dt.float8e4

NW = C * KS * KS  # 1152 weight elements per out-channel row
NWH = NW // 2  # ic-half of the weight row (576)
HALF = (H // 2) * W  # first-half image elements (28 rows)
QTR = (H // 4) * W  # quarter image elements (14 rows)


def build_kernel(rv_vals):
    """Build the single-core Bass module (SPMD: same program on all 8 cores).

    rv_vals: the 5 rv scalars, baked as immediates into the weight-gen ops.
    """
    # Bacc (not plain Bass): its compile() pass pipeline legalizes sync waits
    # (TRN2 allows at most 1 embedded wait per engine instruction; excess waits
    # are split into InstEventSemaphore via generate_event_semaphores).
    nc = Bacc()
    x_p = nc.declare_dram_parameter("x", [BPC, C, H, W], F32, isOutput=False)
    m_p = nc.declare_dram_parameter("M", [C, C, KS, KS], F32, isOutput=False)
    z_p = nc.declare_dram_parameter("Z", [K, C, C, KS, KS], F32, isOutput=False)
    a_p = nc.declare_dram_parameter("Alpha", [C, 1, 1], F32, isOutput=False)
    rv_p = nc.declare_dram_parameter("rv", [1, K], F32, isOutput=False)
    out_p = nc.declare_dram_parameter("out", [BPC, C, H, W], F16, isOutput=True)

    with tile.TileContext(nc) as tc, ExitStack() as ctx:
        const = ctx.enter_context(tc.tile_pool(name="const", bufs=1))
        wg = ctx.enter_context(tc.tile_pool(name="wg", bufs=1))
        zpool = ctx.enter_context(tc.tile_pool(name="zpool", bufs=1))
        xin = ctx.enter_context(tc.tile_pool(name="xin", bufs=4))
        pad = ctx.enter_context(tc.tile_pool(name="pad", bufs=4))
        opool = ctx.enter_context(tc.tile_pool(name="opool", bufs=3))
        ps_t = ctx.enter_context(tc.tile_pool(name="ps_t", bufs=2, space="PSUM"))
        ps_c = ctx.enter_context(tc.tile_pool(name="ps_c", bufs=6, space="PSUM"))

        # ---- constants ----
        # Anti-diagonal permutation: transpose against it yields the transposed
        # tap with REVERSED out-channel columns, which is exactly the column
        # order DoubleRowSwInterleave's weight layout wants.
        identity = const.tile([C, C], BF16)
        nc.gpsimd.memset(identity[:], 0.0)
        nc.gpsimd.affine_select(
            out=identity[:],
            in_=identity[:],
            compare_op=mybir.AluOpType.not_equal,
            fill=1.0,
            base=-(C - 1),
            pattern=[[1, C]],
            channel_multiplier=1,
        )
        # Alpha/rv ride the SWDGE (GpSimd) ring: tiny loads, and the SP/ACT
        # HWDGE rings' first slots belong to x0/M/Z.
        alpha_sb = const.tile([C, 1], F32)
        nc.gpsimd.dma_start(alpha_sb[:], a_p[:].rearrange("c a b -> c (a b)"))
        rv_sb = const.tile([1, K], F32)
        nc.gpsimd.dma_start(rv_sb[:], rv_p[:])

        x_ap = x_p[:]
        o_ap = out_p[:]

        # ---- startup DMA schedule ----
        # Both HWDGE rings drain FIFO; each bulk tensor is split in half with
        # one half per ring so both halves stream concurrently (two issue
        # engines keep the DMA queues fed).  Wire order per ring: the first
        # quarter of image 0 (so its sign() runs early), then M and the Z
        # tensors (the weight chain is the startup critical path), then the
        # rest of image 0 quarter-by-quarter (each quarter split across both
        # rings so its sign can chase the conv chunk cadence), then images
        # 1..7.
        x_sb_0 = xin.tile([C, H * W], F32, tag="x_sb")
        xr0 = x_ap[0].rearrange("c h w -> c (h w)")
        EIG = QTR // 2  # eighth of an image (7 rows)
        nc.sync.dma_start(x_sb_0[:, 0:EIG], xr0[:, 0:EIG])
        nc.scalar.dma_start(x_sb_0[:, EIG:QTR], xr0[:, EIG:QTR])

        m_sb = wg.tile([C, NW], F32)
        mr = m_p[:].rearrange("o i a b -> o (i a b)")
        nc.sync.dma_start(m_sb[:, 0:NWH], mr[:, 0:NWH])
        nc.scalar.dma_start(m_sb[:, NWH:], mr[:, NWH:])

        z_sbs = []
        for k in range(K):
            z_sbs.append(zpool.tile([C, NW], F32, name=f"z{k}", tag=f"z{k}"))
            zr = z_p[k].rearrange("o i a b -> o (i a b)")
            nc.sync.dma_start(z_sbs[k][:, 0:NWH], zr[:, 0:NWH])
            nc.scalar.dma_start(z_sbs[k][:, NWH:], zr[:, NWH:])

        for q in range(1, 4):
            lo = q * QTR
            nc.sync.dma_start(x_sb_0[:, lo : lo + EIG], xr0[:, lo : lo + EIG])
            nc.scalar.dma_start(
                x_sb_0[:, lo + EIG : lo + QTR], xr0[:, lo + EIG : lo + QTR]
            )

        # ---- weight generation: w = M + sum_k rv_k Z_k, per ic-half ----
        # Each half's chain runs on DVE as soon as its z_k half lands; M is
        # folded into the k=0 op.  Interleaving a/b ops lets DVE track the
        # two concurrent DMA streams.
        w_sb = wg.tile([C, NW], F32)
        bw_sb = wg.tile([C, NW], BF16)
        halves = (slice(0, NWH), slice(NWH, NW))
        for s in halves:
            nc.vector.scalar_tensor_tensor(
                w_sb[:, s],
                z_sbs[0][:, s],
                float(rv_vals[0]),
                m_sb[:, s],
                mybir.AluOpType.mult,
                mybir.AluOpType.add,
            )
        for k in range(1, K):
            for s in halves:
                nc.vector.scalar_tensor_tensor(
                    w_sb[:, s],
                    z_sbs[k][:, s],
                    float(rv_vals[k]),
                    w_sb[:, s],
                    mybir.AluOpType.mult,
                    mybir.AluOpType.add,
                )
        for s in halves:
            nc.scalar.sign(bw_sb[:, s], w_sb[:, s])

        # Transpose each tap's [oc, ic] into [ic, oc-reversed] (via the
        # anti-diagonal permutation), then interleave tap pairs column-wise as
        # fp8e4 (+-1 exact): [A127 B127 A126 B126 ... A0 B0] per partition --
        # the DoubleRowSwInterleave weight layout.  Pre-interleaving makes
        # LDWEIGHTS a single 128-column pass instead of DoubleRow's 256-column
        # reload.  Tap 9 (pair 4, slot B) stays all-zero.
        wt = const.tile([C, 5, 2 * C], F8)
        nc.gpsimd.memset(wt[:, 4, :], 0.0)
        bw_r = bw_sb[:].rearrange("o (i j) -> o i j", j=KS * KS)

        def wt_step(j):
            """Transpose tap j and copy it into its interleaved wt slot."""
            tp = ps_t.tile([C, C], BF16)
            nc.tensor.transpose(tp[:], bw_r[:, :, j], identity[:])
            pair, slot = divmod(j, 2)
            wt_h = wt[:].tensor
            dst = bass.AP(wt_h, pair * 2 * C + slot, [[5 * 2 * C, C], [2, C]])
            nc.vector.tensor_copy(dst, tp[:])

        # rv reaches the kernel as baked immediates; touch the tensor so the
        # bound input isn't dead.
        nc.vector.tensor_copy(w_sb[0:1, 0:K], rv_sb[0:1, :])

        def tap_off(r0, j):
            # flat offset of (out-row r0, tap j)'s top-left read in the padded image
            if j == KS * KS:  # zero tap: alias tap 8's window (weights are 0)
                j = KS * KS - 1
            return (r0 + j // KS) * PW + (j % KS)

        def load_sign(i):
            """Image load + binarize into a fresh zero-bordered pad tile."""
            if i == 0:
                x_sb = x_sb_0
            else:
                x_sb = xin.tile([C, H * W], F32, tag="x_sb")
                xr = x_ap[i].rearrange("c h w -> c (h w)")
                nc.sync.dma_start(x_sb[:, 0:HALF], xr[:, 0:HALF])
                nc.scalar.dma_start(x_sb[:, HALF:], xr[:, HALF:])
            ba = pad.tile([C, PH * PW + 2], F8, tag="ba")
            ba_r = ba[:, 0 : PH * PW].rearrange("c (h w) -> c h w", w=PW)
            # Zero only the pad border (sign() fills the interior); GpSimd is
            # idle here and keeps these off the DVE eviction path.
            nc.gpsimd.memset(ba[:, 0:PW], 0.0)
            nc.gpsimd.memset(ba[:, (PH - 1) * PW : PH * PW + 2], 0.0)
            nc.gpsimd.memset(ba_r[:, 1 : H + 1, 0:1], 0.0)
            nc.gpsimd.memset(ba_r[:, 1 : H + 1, W + 1 : PW], 0.0)
            x_r = x_sb[:].rearrange("c (h w) -> c h w", w=W)
            if i == 0:
                # four signs so each quarter runs as soon as its DMA lands
                for q in range(4):
                    r0, r1 = q * (H // 4), (q + 1) * (H // 4)
                    nc.scalar.sign(
                        ba_r[:, r0 + 1 : r1 + 1, 1 : W + 1], x_r[:, r0:r1]
                    )
            else:
                nc.scalar.sign(ba_r[:, 1 : H // 2 + 1, 1 : W + 1], x_r[:, : H // 2])
                nc.scalar.sign(ba_r[:, H // 2 + 1 : H + 1, 1 : W + 1], x_r[:, H // 2 :])
            return ba

        def conv_store(i, ba):
            """9-tap binary conv via 5 DoubleRow matmuls per chunk + eviction.

            For image 0 the 9 weight transposes are interleaved with chunk 0's
            matmuls (pair p's matmul is emitted right after its two taps land
            in wt), so the conv starts as soon as the first weight pair is
            ready instead of after all nine transposes.
            """
            o_sb = opool.tile([C, H * W], F16, tag="o_sb")
            for ch in range(N_CHUNKS):
                pt = ps_c.tile([C, FREE_R], F32, tag="pt")
                r0 = ch * CHUNK_ROWS
                for p in range(5):
                    if i == 0 and ch == 0:
                        for j in (2 * p, 2 * p + 1):
                            if j < KS * KS:
                                wt_step(j)
                    o0 = tap_off(r0, 2 * p)
                    o1 = tap_off(r0, 2 * p + 1)
                    rhs = bass.AP(
                        ba[:].tensor,
                        o0,
                        [[PH * PW + 2, C], [o1 - o0, 2], [1, FREE_R]],
                    )
                    nc.tensor.matmul(
                        pt[:],
                        wt[:, p, :],
                        rhs,
                        start=(p == 0),
                        stop=(p == 4),
                        perf_mode=mybir.MatmulPerfMode.DoubleRowSwInterleave,
                    )
                # PSUM -> SBUF eviction with per-channel Alpha scale on DVE,
                # casting to fp16 and skipping the 2 garbage columns per row.
                nc.vector.tensor_scalar_mul(
                    o_sb[:, ch * FREE : (ch + 1) * FREE].rearrange(
                        "c (a b) -> c a b", b=W
                    ),
                    pt[:].rearrange("c (a b) -> c a b", b=PW)[:, :, 0:W],
                    alpha_sb[:, 0:1],
                )
                # Output DMAs ride the SWDGE (GpSimd) ring: an output DMA
                # waiting on evictions would head-of-line block later input
                # loads on the HWDGE rings.  Half-image granularity shrinks
                # the end-of-kernel tail.  Images >= 6 finish after every
                # input load has drained, so their outputs ride the idle SP
                # HWDGE ring (cheaper issue, no head-of-line risk anymore).
                out_dma = nc.sync.dma_start if i >= 6 else nc.gpsimd.dma_start
                last_img = i == BPC - 1
                if last_img:
                    # per-chunk pieces all the way through the final image:
                    # the drain tail is one eviction + one small DMA.  The
                    # final piece rides the (idle) ACT ring so it doesn't
                    # queue behind earlier pieces on the SP ring.
                    (nc.scalar.dma_start if ch == N_CHUNKS - 1 else out_dma)(
                        o_ap[i].rearrange("c h w -> c (h w)")[
                            :, ch * FREE : (ch + 1) * FREE
                        ],
                        o_sb[:, ch * FREE : (ch + 1) * FREE],
                    )
                elif ch == 3:
                    out_dma(
                        o_ap[i].rearrange("c h w -> c (h w)")[:, 0 : 4 * FREE],
                        o_sb[:, 0 : 4 * FREE],
                    )
                elif ch == N_CHUNKS - 1:
                    out_dma(
                        o_ap[i].rearrange("c h w -> c (h w)")[:, 4 * FREE :],
                        o_sb[:, 4 * FREE :],
                    )

        # Software-pipelined: image i+1's load/sign issues before image i's
        # conv+store so ScalarE signs (and input DMAs) always run ahead.
        prev_ba = None
        for i in range(BPC):
            ba = load_sign(i)
            if prev_ba is not None:
                conv_store(i - 1, prev_ba)
            prev_ba = ba
        conv_store(BPC - 1, prev_ba)

    nc.finalize()
    return nc


_CACHE = {}


def _get_nc(rv):
    key = rv.tobytes()
    if key not in _CACHE:
        _CACHE[key] = build_kernel(np.asarray(rv, np.float32).reshape(-1))
    return _CACHE[key]


def _run(inputs, trace=False):
    x = np.ascontiguousarray(np.asarray(inputs["x"], np.float32))
    M = np.ascontiguousarray(np.asarray(inputs["M"], np.float32))
    Z = np.ascontiguousarray(np.asarray(inputs["Z"], np.float32))
    Alpha = np.ascontiguousarray(np.asarray(inputs["Alpha"], np.float32))
    rv = np.ascontiguousarray(np.asarray(inputs["rv"], np.float32))
    nc = _get_nc(rv)
    in_maps = [
        {"x": x[c * BPC : (c + 1) * BPC], "M": M, "Z": Z, "Alpha": Alpha, "rv": rv}
        for c in range(N_CORES)
    ]
    res = run_bass_kernel_spmd(nc, in_maps, list(range(N_CORES)), trace=trace)
    out = np.concatenate(
        [res.results[c]["out"] for c in range(N_CORES)], axis=0
    ).astype(np.float32)
    return out, res


def kernel(**inputs):
    out, _ = _run(inputs, trace=False)
    return out


def kernel_traced(**inputs):
    out, res = _run(inputs, trace=True)
    return out, res


# revision 7
# speedup vs baseline: 1.1496x; 1.1496x over previous
"""BinarizeConv2dSDP kernel for Trainium2 (8 NeuronCores, data-parallel over batch).

out = conv2d(sign(x), sign(M + sum_k rv[k] * Z[k]), stride 1, pad 1) * Alpha

Key simplification: the reference normalizes (M, Z) by rsqrt(M^2 + sum Z^2 / SCALE)
before forming w = rv@Z + M, but that factor is strictly positive and applied
multiplicatively to the whole expression, so sign(w) is unaffected.  The binary
weights are just sign(M + sum_k rv[k] Z[k]).

Strategy per core (8 images each):
  - every bulk HBM load is split into two halves issued on the SP (sync) and
    ACT (scalar) HWDGE rings concurrently: two issue engines keep the 16 DMA
    queues fed (~1.4x the effective startup bandwidth of a single FIFO ring),
    and the weight stream (M, Z) goes out ahead of most of image 0 so the
    weight-gen chain finishes early.
  - weight gen on DVE per ic-half: w_h = (rv0*z0_h + M_h), then += rv_k z_k_h,
    sign -> bf16 per half on ACT as soon as that half's chain is done.
    M is folded into the first scalar_tensor_tensor so no separate add.
  - 9 PE transposes (against an anti-diagonal permutation) produce the
    column-reversed, pair-interleaved fp8e4 weight layout that
    DoubleRowSwInterleave expects (plus one all-zero tap so 9 taps = 5 pairs).
  - conv: 5 fp8 DoubleRowSwInterleave matmuls per 8-row chunk (2 taps per
    matmul, K=256 effective contraction), accumulated in PSUM over a 58-wide
    zero-padded sign(x) image; the free dim spans whole padded rows (464) so
    the moving AP stays 3D, leaving 2 garbage columns per row that the
    eviction skips.
  - Alpha scaling on DVE during PSUM->SBUF eviction, emitting float16
    (exact conv integers * f32 alpha rounded to fp16: rel err ~5e-4, well
    inside the 2e-2 gate); the host upcasts to f32 after the gather.  fp16
    halves the output HBM traffic and the end-of-kernel drain tail.
  - steady-state output DMAs ride the SWDGE (GpSimd) ring so they never
    head-of-line block input loads on the HWDGE rings (the last two images'
    outputs switch to the then-idle SP ring).
"""

import numpy as np
from contextlib import ExitStack

import concourse.bass as bass
import concourse.mybir as mybir
import concourse.tile as tile
from concourse.bacc import Bacc
from concourse.bass_utils import run_bass_kernel_spmd

N_CORES = 8
B, C, H, W = 64, 128, 56, 56
BPC = B // N_CORES  # images per core
KS, K = 3, 5
PH, PW = H + 2, W + 2  # zero-padded image
CHUNK_ROWS = 8
N_CHUNKS = H // CHUNK_ROWS
FREE = CHUNK_ROWS * W  # valid output elements per chunk (448)
FREE_R = CHUNK_ROWS * PW  # matmul free dim incl. garbage cols (464 <= 512)
F32 = mybir.dt.float32
F16 = mybir.dt.float16
BF16 = mybir.dt.bfloat16
F8 = mybir.dt.float8e4

NW = C * KS * KS  # 1152 weight elements per out-channel row
NWH = NW // 2  # ic-half of the weight row (576)
HALF = (H // 2) * W  # first-half image elements (28 rows)
QTR = (H // 4) * W  # quarter image elements (14 rows)


def build_kernel(rv_vals):
    """Build the single-core Bass module (SPMD: same program on all 8 cores).

    rv_vals: the 5 rv scalars, baked as immediates into the weight-gen ops.
    """
    # Bacc (not plain Bass): its compile() pass pipeline legalizes sync waits
    # (TRN2 allows at most 1 embedded wait per engine instruction; excess waits
    # are split into InstEventSemaphore via generate_event_semaphores).
    nc = Bacc()
    x_p = nc.declare_dram_parameter("x", [BPC, C, H, W], F32, isOutput=False)
    m_p = nc.declare_dram_parameter("M", [C, C, KS, KS], F32, isOutput=False)
    z_p = nc.declare_dram_parameter("Z", [K, C, C, KS, KS], F32, isOutput=False)
    a_p = nc.declare_dram_parameter("Alpha", [C, 1, 1], F32, isOutput=False)
    rv_p = nc.declare_dram_parameter("rv", [1, K], F32, isOutput=False)
    out_p = nc.declare_dram_parameter("out", [BPC, C, H, W], F16, isOutput=True)

    with tile.TileContext(nc) as tc, ExitStack() as ctx:
        const = ctx.enter_context(tc.tile_pool(name="const", bufs=1))
        wg = ctx.enter_context(tc.tile_pool(name="wg", bufs=1))
        zpool = ctx.enter_context(tc.tile_pool(name="zpool", bufs=1))
        xin = ctx.enter_context(tc.tile_pool(name="xin", bufs=4))
        pad = ctx.enter_context(tc.tile_pool(name="pad", bufs=4))
        opool = ctx.enter_context(tc.tile_pool(name="opool", bufs=3))
        ps_t = ctx.enter_context(tc.tile_pool(name="ps_t", bufs=2, space="PSUM"))
        ps_c = ctx.enter_context(tc.tile_pool(name="ps_c", bufs=6, space="PSUM"))

        # ---- constants ----
        # Anti-diagonal permutation: transpose against it yields the transposed
        # tap with REVERSED out-channel columns, which is exactly the column
        # order DoubleRowSwInterleave's weight layout wants.
        identity = const.tile([C, C], BF16)
        nc.gpsimd.memset(identity[:], 0.0)
        nc.gpsimd.affine_select(
            out=identity[:],
            in_=identity[:],
            compare_op=mybir.AluOpType.not_equal,
            fill=1.0,
            base=-(C - 1),
            pattern=[[1, C]],
            channel_multiplier=1,
        )
        # Alpha/rv ride the SWDGE (GpSimd) ring: tiny loads, and the SP/ACT
        # HWDGE rings' first slots belong to x0/M/Z.
        alpha_sb = const.tile([C, 1], F32)
        nc.gpsimd.dma_start(alpha_sb[:], a_p[:].rearrange("c a b -> c (a b)"))
        rv_sb = const.tile([1, K], F32)
        nc.gpsimd.dma_start(rv_sb[:], rv_p[:])

        x_ap = x_p[:]
        o_ap = out_p[:]

        # ---- startup DMA schedule ----
        # Both HWDGE rings drain FIFO; each bulk tensor is split in half with
        # one half per ring so both halves stream concurrently (two issue
        # engines keep the DMA queues fed).  Wire order per ring: the first
        # quarter of image 0 (so its sign() runs early), then M and the Z
        # tensors (the weight chain is the startup critical path), then the
        # rest of image 0 quarter-by-quarter (each quarter split across both
        # rings so its sign can chase the conv chunk cadence), then images
        # 1..7.
        x_sb_0 = xin.tile([C, H * W], F32, tag="x_sb")
        xr0 = x_ap[0].rearrange("c h w -> c (h w)")
        EIG = QTR // 2  # eighth of an image (7 rows)
        # First wave on FOUR rings at once: x0's first quarter rides the
        # otherwise-idle PE and DVE rings, M halves lead the SP/ACT rings.
        # More DMAs in flight during the queue ramp-up, and the weight stream
        # (the startup critical path) isn't delayed by any x0 bytes.
        nc.tensor.dma_start(x_sb_0[:, 0:EIG], xr0[:, 0:EIG])
        nc.vector.dma_start(x_sb_0[:, EIG:QTR], xr0[:, EIG:QTR])

        m_sb = wg.tile([C, NW], F32)
        mr = m_p[:].rearrange("o i a b -> o (i a b)")
        nc.sync.dma_start(m_sb[:, 0:NWH], mr[:, 0:NWH])
        nc.scalar.dma_start(m_sb[:, NWH:], mr[:, NWH:])

        z_sbs = []
        for k in range(K):
            z_sbs.append(zpool.tile([C, NW], F32, name=f"z{k}", tag=f"z{k}"))
            zr = z_p[k].rearrange("o i a b -> o (i a b)")
            nc.sync.dma_start(z_sbs[k][:, 0:NWH], zr[:, 0:NWH])
            nc.scalar.dma_start(z_sbs[k][:, NWH:], zr[:, NWH:])

        for q in range(1, 4):
            lo = q * QTR
            nc.sync.dma_start(x_sb_0[:, lo : lo + EIG], xr0[:, lo : lo + EIG])
            nc.scalar.dma_start(
                x_sb_0[:, lo + EIG : lo + QTR], xr0[:, lo + EIG : lo + QTR]
            )

        # ---- weight generation: w = M + sum_k rv_k Z_k, per ic-half ----
        # Each half's chain runs on DVE as soon as its z_k half lands; M is
        # folded into the k=0 op.  Interleaving a/b ops lets DVE track the
        # two concurrent DMA streams.
        w_sb = wg.tile([C, NW], F32)
        bw_sb = wg.tile([C, NW], BF16)
        halves = (slice(0, NWH), slice(NWH, NW))
        for s in halves:
            nc.vector.scalar_tensor_tensor(
                w_sb[:, s],
                z_sbs[0][:, s],
                float(rv_vals[0]),
                m_sb[:, s],
                mybir.AluOpType.mult,
                mybir.AluOpType.add,
            )
        for k in range(1, K):
            for s in halves:
                nc.vector.scalar_tensor_tensor(
                    w_sb[:, s],
                    z_sbs[k][:, s],
                    float(rv_vals[k]),
                    w_sb[:, s],
                    mybir.AluOpType.mult,
                    mybir.AluOpType.add,
                )
        for s in halves:
            nc.scalar.sign(bw_sb[:, s], w_sb[:, s])

        # Transpose each tap's [oc, ic] into [ic, oc-reversed] (via the
        # anti-diagonal permutation), then interleave tap pairs column-wise as
        # fp8e4 (+-1 exact): [A127 B127 A126 B126 ... A0 B0] per partition --
        # the DoubleRowSwInterleave weight layout.  Pre-interleaving makes
        # LDWEIGHTS a single 128-column pass instead of DoubleRow's 256-column
        # reload.  Tap 9 (pair 4, slot B) stays all-zero.
        wt = const.tile([C, 5, 2 * C], F8)
        nc.gpsimd.memset(wt[:, 4, :], 0.0)
        bw_r = bw_sb[:].rearrange("o (i j) -> o i j", j=KS * KS)

        def wt_step(j):
            """Transpose tap j and copy it into its interleaved wt slot."""
            tp = ps_t.tile([C, C], BF16)
            nc.tensor.transpose(tp[:], bw_r[:, :, j], identity[:])
            pair, slot = divmod(j, 2)
            wt_h = wt[:].tensor
            dst = bass.AP(wt_h, pair * 2 * C + slot, [[5 * 2 * C, C], [2, C]])
            nc.vector.tensor_copy(dst, tp[:])

        # rv reaches the kernel as baked immediates; touch the tensor so the
        # bound input isn't dead.
        nc.vector.tensor_copy(w_sb[0:1, 0:K], rv_sb[0:1, :])

        def tap_off(r0, j):
            # flat offset of (out-row r0, tap j)'s top-left read in the padded image
            if j == KS * KS:  # zero tap: alias tap 8's window (weights are 0)
                j = KS * KS - 1
            return (r0 + j // KS) * PW + (j % KS)

        def load_sign(i):
            """Image load + binarize into a fresh zero-bordered pad tile."""
            if i == 0:
                x_sb = x_sb_0
            else:
                x_sb = xin.tile([C, H * W], F32, tag="x_sb")
                xr = x_ap[i].rearrange("c h w -> c (h w)")
                nc.sync.dma_start(x_sb[:, 0:HALF], xr[:, 0:HALF])
                nc.scalar.dma_start(x_sb[:, HALF:], xr[:, HALF:])
            ba = pad.tile([C, PH * PW + 2], F8, tag="ba")
            ba_r = ba[:, 0 : PH * PW].rearrange("c (h w) -> c h w", w=PW)
            # Zero only the pad border (sign() fills the interior); GpSimd is
            # idle here and keeps these off the DVE eviction path.
            nc.gpsimd.memset(ba[:, 0:PW], 0.0)
            nc.gpsimd.memset(ba[:, (PH - 1) * PW : PH * PW + 2], 0.0)
            nc.gpsimd.memset(ba_r[:, 1 : H + 1, 0:1], 0.0)
            nc.gpsimd.memset(ba_r[:, 1 : H + 1, W + 1 : PW], 0.0)
            x_r = x_sb[:].rearrange("c (h w) -> c h w", w=W)
            if i == 0:
                # four signs so each quarter runs as soon as its DMA lands
                for q in range(4):
                    r0, r1 = q * (H // 4), (q + 1) * (H // 4)
                    nc.scalar.sign(
                        ba_r[:, r0 + 1 : r1 + 1, 1 : W + 1], x_r[:, r0:r1]
                    )
            else:
                nc.scalar.sign(ba_r[:, 1 : H // 2 + 1, 1 : W + 1], x_r[:, : H // 2])
                nc.scalar.sign(ba_r[:, H // 2 + 1 : H + 1, 1 : W + 1], x_r[:, H // 2 :])
            return ba

        def conv_store(i, ba):
            """9-tap binary conv via 5 DoubleRow matmuls per chunk + eviction.

            For image 0 the 9 weight transposes are interleaved with chunk 0's
            matmuls (pair p's matmul is emitted right after its two taps land
            in wt), so the conv starts as soon as the first weight pair is
            ready instead of after all nine transposes.
            """
            o_sb = opool.tile([C, H * W], F16, tag="o_sb")
            for ch in range(N_CHUNKS):
                pt = ps_c.tile([C, FREE_R], F32, tag="pt")
                r0 = ch * CHUNK_ROWS
                for p in range(5):
                    if i == 0 and ch == 0:
                        for j in (2 * p, 2 * p + 1):
                            if j < KS * KS:
                                wt_step(j)
                    o0 = tap_off(r0, 2 * p)
                    o1 = tap_off(r0, 2 * p + 1)
                    rhs = bass.AP(
                        ba[:].tensor,
                        o0,
                        [[PH * PW + 2, C], [o1 - o0, 2], [1, FREE_R]],
                    )
                    nc.tensor.matmul(
                        pt[:],
                        wt[:, p, :],
                        rhs,
                        start=(p == 0),
                        stop=(p == 4),
                        perf_mode=mybir.MatmulPerfMode.DoubleRowSwInterleave,
                    )
                # PSUM -> SBUF eviction with per-channel Alpha scale on DVE,
                # casting to fp16 and skipping the 2 garbage columns per row.
                nc.vector.tensor_scalar_mul(
                    o_sb[:, ch * FREE : (ch + 1) * FREE].rearrange(
                        "c (a b) -> c a b", b=W
                    ),
                    pt[:].rearrange("c (a b) -> c a b", b=PW)[:, :, 0:W],
                    alpha_sb[:, 0:1],
                )
                # Output DMAs ride the SWDGE (GpSimd) ring: an output DMA
                # waiting on evictions would head-of-line block later input
                # loads on the HWDGE rings.  Half-image granularity shrinks
                # the end-of-kernel tail.  Images >= 6 finish after every
                # input load has drained, so their outputs ride the idle SP
                # HWDGE ring (cheaper issue, no head-of-line risk anymore).
                out_dma = nc.sync.dma_start if i >= 6 else nc.gpsimd.dma_start
                last_img = i == BPC - 1
                if last_img:
                    # per-chunk pieces all the way through the final image:
                    # the drain tail is one eviction + one small DMA.  The
                    # final piece rides the (idle) ACT ring so it doesn't
                    # queue behind earlier pieces on the SP ring.
                    (nc.scalar.dma_start if ch == N_CHUNKS - 1 else out_dma)(
                        o_ap[i].rearrange("c h w -> c (h w)")[
                            :, ch * FREE : (ch + 1) * FREE
                        ],
                        o_sb[:, ch * FREE : (ch + 1) * FREE],
                    )
                elif ch == 3:
                    out_dma(
                        o_ap[i].rearrange("c h w -> c (h w)")[:, 0 : 4 * FREE],
                        o_sb[:, 0 : 4 * FREE],
                    )
                elif ch == N_CHUNKS - 1:
                    out_dma(
                        o_ap[i].rearrange("c h w -> c (h w)")[:, 4 * FREE :],
                        o_sb[:, 4 * FREE :],
                    )

        # Software-pipelined: image i+1's load/sign issues before image i's
        # conv+store so ScalarE signs (and input DMAs) always run ahead.
        prev_ba = None
        for i in range(BPC):
            ba = load_sign(i)
            if prev_ba is not None:
                conv_store(i - 1, prev_ba)
            prev_ba = ba
        conv_store(BPC - 1, prev_ba)

    nc.finalize()
    return nc


_CACHE = {}


def _get_nc(rv):
    key = rv.tobytes()
    if key not in _CACHE:
        _CACHE[key] = build_kernel(np.asarray(rv, np.float32).reshape(-1))
    return _CACHE[key]


def _run(inputs, trace=False):
    x = np.ascontiguousarray(np.asarray(inputs["x"], np.float32))
    M = np.ascontiguousarray(np.asarray(inputs["M"], np.float32))
    Z = np.ascontiguousarray(np.asarray(inputs["Z"], np.float32))
    Alpha = np.ascontiguousarray(np.asarray(inputs["Alpha"], np.float32))
    rv = np.ascontiguousarray(np.asarray(inputs["rv"], np.float32))
    nc = _get_nc(rv)
    in_maps = [
        {"x": x[c * BPC : (c + 1) * BPC], "M": M, "Z": Z, "Alpha": Alpha, "rv": rv}
        for c in range(N_CORES)
    ]
    res = run_bass_kernel_spmd(nc, in_maps, list(range(N_CORES)), trace=trace)
    out = np.concatenate(
        [res.results[c]["out"] for c in range(N_CORES)], axis=0
    ).astype(np.float32)
    return out, res


def kernel(**inputs):
    out, _ = _run(inputs, trace=False)
    return out


def kernel_traced(**inputs):
    out, res = _run(inputs, trace=True)
    return out, res


# revision 8
# speedup vs baseline: 1.1502x; 1.0005x over previous
"""BinarizeConv2dSDP kernel for Trainium2 (8 NeuronCores, data-parallel over batch).

out = conv2d(sign(x), sign(M + sum_k rv[k] * Z[k]), stride 1, pad 1) * Alpha

Key simplification: the reference normalizes (M, Z) by rsqrt(M^2 + sum Z^2 / SCALE)
before forming w = rv@Z + M, but that factor is strictly positive and applied
multiplicatively to the whole expression, so sign(w) is unaffected.  The binary
weights are just sign(M + sum_k rv[k] Z[k]).

Strategy per core (8 images each):
  - every bulk HBM load is split into two halves issued on the SP (sync) and
    ACT (scalar) HWDGE rings concurrently: two issue engines keep the 16 DMA
    queues fed (~1.4x the effective startup bandwidth of a single FIFO ring),
    and the weight stream (M, Z) goes out ahead of most of image 0 so the
    weight-gen chain finishes early.
  - weight gen on DVE per ic-half: w_h = (rv0*z0_h + M_h), then += rv_k z_k_h,
    sign -> bf16 per half on ACT as soon as that half's chain is done.
    M is folded into the first scalar_tensor_tensor so no separate add.
  - 9 PE transposes (against an anti-diagonal permutation) produce the
    column-reversed, pair-interleaved fp8e4 weight layout that
    DoubleRowSwInterleave expects (plus one all-zero tap so 9 taps = 5 pairs).
  - conv: 5 fp8 DoubleRowSwInterleave matmuls per 8-row chunk (2 taps per
    matmul, K=256 effective contraction), accumulated in PSUM over a 58-wide
    zero-padded sign(x) image; the free dim spans whole padded rows (464) so
    the moving AP stays 3D, leaving 2 garbage columns per row that the
    eviction skips.
  - Alpha scaling on DVE during PSUM->SBUF eviction, emitting float16
    (exact conv integers * f32 alpha rounded to fp16: rel err ~5e-4, well
    inside the 2e-2 gate); the host upcasts to f32 after the gather.  fp16
    halves the output HBM traffic and the end-of-kernel drain tail.
  - steady-state output DMAs ride the SWDGE (GpSimd) ring so they never
    head-of-line block input loads on the HWDGE rings (the last two images'
    outputs switch to the then-idle SP ring).
"""

import numpy as np
from contextlib import ExitStack

import concourse.bass as bass
import concourse.mybir as mybir
import concourse.tile as tile
from concourse.bacc import Bacc
from concourse.bass_utils import run_bass_kernel_spmd

N_CORES = 8
B, C, H, W = 64, 128, 56, 56
BPC = B // N_CORES  # images per core
KS, K = 3, 5
PH, PW = H + 2, W + 2  # zero-padded image
CHUNK_ROWS = 8
N_CHUNKS = H // CHUNK_ROWS
FREE = CHUNK_ROWS * W  # valid output elements per chunk (448)
FREE_R = CHUNK_ROWS * PW  # matmul free dim incl. garbage cols (464 <= 512)
F32 = mybir.dt.float32
F16 = mybir.dt.float16
BF16 = mybir.dt.bfloat16
F8 = mybir.dt.float8e4

NW = C * KS * KS  # 1152 weight elements per out-channel row
NWH = NW // 2  # ic-half of the weight row (576)
HALF = (H // 2) * W  # first-half image elements (28 rows)
QTR = (H // 4) * W  # quarter image elements (14 rows)


def build_kernel(rv_vals):
    """Build the single-core Bass module (SPMD: same program on all 8 cores).

    rv_vals: the 5 rv scalars, baked as immediates into the weight-gen ops.
    """
    # Bacc (not plain Bass): its compile() pass pipeline legalizes sync waits
    # (TRN2 allows at most 1 embedded wait per engine instruction; excess waits
    # are split into InstEventSemaphore via generate_event_semaphores).
    nc = Bacc()
    x_p = nc.declare_dram_parameter("x", [BPC, C, H, W], F32, isOutput=False)
    m_p = nc.declare_dram_parameter("M", [C, C, KS, KS], F32, isOutput=False)
    z_p = nc.declare_dram_parameter("Z", [K, C, C, KS, KS], F32, isOutput=False)
    a_p = nc.declare_dram_parameter("Alpha", [C, 1, 1], F32, isOutput=False)
    rv_p = nc.declare_dram_parameter("rv", [1, K], F32, isOutput=False)
    out_p = nc.declare_dram_parameter("out", [BPC, C, H, W], F16, isOutput=True)

    with tile.TileContext(nc) as tc, ExitStack() as ctx:
        const = ctx.enter_context(tc.tile_pool(name="const", bufs=1))
        wg = ctx.enter_context(tc.tile_pool(name="wg", bufs=1))
        zpool = ctx.enter_context(tc.tile_pool(name="zpool", bufs=1))
        xin = ctx.enter_context(tc.tile_pool(name="xin", bufs=4))
        pad = ctx.enter_context(tc.tile_pool(name="pad", bufs=4))
        opool = ctx.enter_context(tc.tile_pool(name="opool", bufs=3))
        ps_t = ctx.enter_context(tc.tile_pool(name="ps_t", bufs=2, space="PSUM"))
        ps_c = ctx.enter_context(tc.tile_pool(name="ps_c", bufs=6, space="PSUM"))

        # ---- constants ----
        # Anti-diagonal permutation: transpose against it yields the transposed
        # tap with REVERSED out-channel columns, which is exactly the column
        # order DoubleRowSwInterleave's weight layout wants.
        identity = const.tile([C, C], BF16)
        nc.gpsimd.memset(identity[:], 0.0)
        nc.gpsimd.affine_select(
            out=identity[:],
            in_=identity[:],
            compare_op=mybir.AluOpType.not_equal,
            fill=1.0,
            base=-(C - 1),
            pattern=[[1, C]],
            channel_multiplier=1,
        )
        # Alpha/rv ride the SWDGE (GpSimd) ring: tiny loads, and the SP/ACT
        # HWDGE rings' first slots belong to x0/M/Z.
        alpha_sb = const.tile([C, 1], F32)
        nc.gpsimd.dma_start(alpha_sb[:], a_p[:].rearrange("c a b -> c (a b)"))
        rv_sb = const.tile([1, K], F32)
        nc.gpsimd.dma_start(rv_sb[:], rv_p[:])

        x_ap = x_p[:]
        o_ap = out_p[:]

        # ---- startup DMA schedule ----
        # Both HWDGE rings drain FIFO; each bulk tensor is split in half with
        # one half per ring so both halves stream concurrently (two issue
        # engines keep the DMA queues fed).  Wire order per ring: the first
        # quarter of image 0 (so its sign() runs early), then M and the Z
        # tensors (the weight chain is the startup critical path), then the
        # rest of image 0 quarter-by-quarter (each quarter split across both
        # rings so its sign can chase the conv chunk cadence), then images
        # 1..7.
        x_sb_0 = xin.tile([C, H * W], F32, tag="x_sb")
        xr0 = x_ap[0].rearrange("c h w -> c (h w)")
        EIG = QTR // 2  # eighth of an image (7 rows)
        # First wave on THREE rings at once: x0's first quarter rides the
        # otherwise-idle SWDGE (GpSimd) ring, M halves lead the SP/ACT rings.
        # More DMAs in flight during the queue ramp-up, and the weight stream
        # (the startup critical path) isn't delayed by any x0 bytes.
        nc.gpsimd.dma_start(x_sb_0[:, 0:EIG], xr0[:, 0:EIG])
        nc.gpsimd.dma_start(x_sb_0[:, EIG:QTR], xr0[:, EIG:QTR])

        m_sb = wg.tile([C, NW], F32)
        mr = m_p[:].rearrange("o i a b -> o (i a b)")
        nc.sync.dma_start(m_sb[:, 0:NWH], mr[:, 0:NWH])
        nc.scalar.dma_start(m_sb[:, NWH:], mr[:, NWH:])

        z_sbs = []
        for k in range(K):
            z_sbs.append(zpool.tile([C, NW], F32, name=f"z{k}", tag=f"z{k}"))
            zr = z_p[k].rearrange("o i a b -> o (i a b)")
            nc.sync.dma_start(z_sbs[k][:, 0:NWH], zr[:, 0:NWH])
            nc.scalar.dma_start(z_sbs[k][:, NWH:], zr[:, NWH:])

        for q in range(1, 4):
            lo = q * QTR
            nc.sync.dma_start(x_sb_0[:, lo : lo + EIG], xr0[:, lo : lo + EIG])
            nc.scalar.dma_start(
                x_sb_0[:, lo + EIG : lo + QTR], xr0[:, lo + EIG : lo + QTR]
            )

        # ---- weight generation: w = M + sum_k rv_k Z_k, per ic-half ----
        # Each half's chain runs on DVE as soon as its z_k half lands; M is
        # folded into the k=0 op.  Interleaving a/b ops lets DVE track the
        # two concurrent DMA streams.
        w_sb = wg.tile([C, NW], F32)
        bw_sb = wg.tile([C, NW], BF16)
        halves = (slice(0, NWH), slice(NWH, NW))
        for s in halves:
            nc.vector.scalar_tensor_tensor(
                w_sb[:, s],
                z_sbs[0][:, s],
                float(rv_vals[0]),
                m_sb[:, s],
                mybir.AluOpType.mult,
                mybir.AluOpType.add,
            )
        for k in range(1, K):
            for s in halves:
                nc.vector.scalar_tensor_tensor(
                    w_sb[:, s],
                    z_sbs[k][:, s],
                    float(rv_vals[k]),
                    w_sb[:, s],
                    mybir.AluOpType.mult,
                    mybir.AluOpType.add,
                )
        for s in halves:
            nc.scalar.sign(bw_sb[:, s], w_sb[:, s])

        # Transpose each tap's [oc, ic] into [ic, oc-reversed] (via the
        # anti-diagonal permutation), then interleave tap pairs column-wise as
        # fp8e4 (+-1 exact): [A127 B127 A126 B126 ... A0 B0] per partition --
        # the DoubleRowSwInterleave weight layout.  Pre-interleaving makes
        # LDWEIGHTS a single 128-column pass instead of DoubleRow's 256-column
        # reload.  Tap 9 (pair 4, slot B) stays all-zero.
        wt = const.tile([C, 5, 2 * C], F8)
        nc.gpsimd.memset(wt[:, 4, :], 0.0)
        bw_r = bw_sb[:].rearrange("o (i j) -> o i j", j=KS * KS)

        def wt_step(j):
            """Transpose tap j and copy it into its interleaved wt slot."""
            tp = ps_t.tile([C, C], BF16)
            nc.tensor.transpose(tp[:], bw_r[:, :, j], identity[:])
            pair, slot = divmod(j, 2)
            wt_h = wt[:].tensor
            dst = bass.AP(wt_h, pair * 2 * C + slot, [[5 * 2 * C, C], [2, C]])
            nc.vector.tensor_copy(dst, tp[:])

        # rv reaches the kernel as baked immediates; touch the tensor so the
        # bound input isn't dead.
        nc.vector.tensor_copy(w_sb[0:1, 0:K], rv_sb[0:1, :])

        def tap_off(r0, j):
            # flat offset of (out-row r0, tap j)'s top-left read in the padded image
            if j == KS * KS:  # zero tap: alias tap 8's window (weights are 0)
                j = KS * KS - 1
            return (r0 + j // KS) * PW + (j % KS)

        def load_sign(i):
            """Image load + binarize into a fresh zero-bordered pad tile."""
            if i == 0:
                x_sb = x_sb_0
            else:
                x_sb = xin.tile([C, H * W], F32, tag="x_sb")
                xr = x_ap[i].rearrange("c h w -> c (h w)")
                nc.sync.dma_start(x_sb[:, 0:HALF], xr[:, 0:HALF])
                nc.scalar.dma_start(x_sb[:, HALF:], xr[:, HALF:])
            ba = pad.tile([C, PH * PW + 2], F8, tag="ba")
            ba_r = ba[:, 0 : PH * PW].rearrange("c (h w) -> c h w", w=PW)
            # Zero only the pad border (sign() fills the interior); GpSimd is
            # idle here and keeps these off the DVE eviction path.
            nc.gpsimd.memset(ba[:, 0:PW], 0.0)
            nc.gpsimd.memset(ba[:, (PH - 1) * PW : PH * PW + 2], 0.0)
            nc.gpsimd.memset(ba_r[:, 1 : H + 1, 0:1], 0.0)
            nc.gpsimd.memset(ba_r[:, 1 : H + 1, W + 1 : PW], 0.0)
            x_r = x_sb[:].rearrange("c (h w) -> c h w", w=W)
            if i == 0:
                # four signs so each quarter runs as soon as its DMA lands
                for q in range(4):
                    r0, r1 = q * (H // 4), (q + 1) * (H // 4)
                    nc.scalar.sign(
                        ba_r[:, r0 + 1 : r1 + 1, 1 : W + 1], x_r[:, r0:r1]
                    )
            else:
                nc.scalar.sign(ba_r[:, 1 : H // 2 + 1, 1 : W + 1], x_r[:, : H // 2])
                nc.scalar.sign(ba_r[:, H // 2 + 1 : H + 1, 1 : W + 1], x_r[:, H // 2 :])
            return ba

        def conv_store(i, ba):
            """9-tap binary conv via 5 DoubleRow matmuls per chunk + eviction.

            For image 0 the 9 weight transposes are interleaved with chunk 0's
            matmuls (pair p's matmul is emitted right after its two taps land
            in wt), so the conv starts as soon as the first weight pair is
            ready instead of after all nine transposes.
            """
            o_sb = opool.tile([C, H * W], F16, tag="o_sb")
            for ch in range(N_CHUNKS):
                pt = ps_c.tile([C, FREE_R], F32, tag="pt")
                r0 = ch * CHUNK_ROWS
                for p in range(5):
                    if i == 0 and ch == 0:
                        for j in (2 * p, 2 * p + 1):
                            if j < KS * KS:
                                wt_step(j)
                    o0 = tap_off(r0, 2 * p)
                    o1 = tap_off(r0, 2 * p + 1)
                    rhs = bass.AP(
                        ba[:].tensor,
                        o0,
                        [[PH * PW + 2, C], [o1 - o0, 2], [1, FREE_R]],
                    )
                    nc.tensor.matmul(
                        pt[:],
                        wt[:, p, :],
                        rhs,
                        start=(p == 0),
                        stop=(p == 4),
                        perf_mode=mybir.MatmulPerfMode.DoubleRowSwInterleave,
                    )
                # PSUM -> SBUF eviction with per-channel Alpha scale on DVE,
                # casting to fp16 and skipping the 2 garbage columns per row.
                nc.vector.tensor_scalar_mul(
                    o_sb[:, ch * FREE : (ch + 1) * FREE].rearrange(
                        "c (a b) -> c a b", b=W
                    ),
                    pt[:].rearrange("c (a b) -> c a b", b=PW)[:, :, 0:W],
                    alpha_sb[:, 0:1],
                )
                # Output DMAs ride the SWDGE (GpSimd) ring: an output DMA
                # waiting on evictions would head-of-line block later input
                # loads on the HWDGE rings.  Half-image granularity shrinks
                # the end-of-kernel tail.  Images >= 6 finish after every
                # input load has drained, so their outputs ride the idle SP
                # HWDGE ring (cheaper issue, no head-of-line risk anymore).
                out_dma = nc.sync.dma_start if i >= 6 else nc.gpsimd.dma_start
                last_img = i == BPC - 1
                if last_img:
                    # per-chunk pieces all the way through the final image:
                    # the drain tail is one eviction + one small DMA.  The
                    # final piece rides the (idle) ACT ring so it doesn't
                    # queue behind earlier pieces on the SP ring.
                    (nc.scalar.dma_start if ch == N_CHUNKS - 1 else out_dma)(
                        o_ap[i].rearrange("c h w -> c (h w)")[
                            :, ch * FREE : (ch + 1) * FREE
                        ],
                        o_sb[:, ch * FREE : (ch + 1) * FREE],
                    )
                elif ch == 3:
                    out_dma(
                        o_ap[i].rearrange("c h w -> c (h w)")[:, 0 : 4 * FREE],
                        o_sb[:, 0 : 4 * FREE],
                    )
                elif ch == N_CHUNKS - 1:
                    out_dma(
                        o_ap[i].rearrange("c h w -> c (h w)")[:, 4 * FREE :],
                        o_sb[:, 4 * FREE :],
                    )

        # Software-pipelined: image i+1's load/sign issues before image i's
        # conv+store so ScalarE signs (and input DMAs) always run ahead.
        prev_ba = None
        for i in range(BPC):
            ba = load_sign(i)
            if prev_ba is not None:
                conv_store(i - 1, prev_ba)
            prev_ba = ba
        conv_store(BPC - 1, prev_ba)

    nc.finalize()
    return nc


_CACHE = {}


def _get_nc(rv):
    key = rv.tobytes()
    if key not in _CACHE:
        _CACHE[key] = build_kernel(np.asarray(rv, np.float32).reshape(-1))
    return _CACHE[key]


def _run(inputs, trace=False):
    x = np.ascontiguousarray(np.asarray(inputs["x"], np.float32))
    M = np.ascontiguousarray(np.asarray(inputs["M"], np.float32))
    Z = np.ascontiguousarray(np.asarray(inputs["Z"], np.float32))
    Alpha = np.ascontiguousarray(np.asarray(inputs["Alpha"], np.float32))
    rv = np.ascontiguousarray(np.asarray(inputs["rv"], np.float32))
    nc = _get_nc(rv)
    in_maps = [
        {"x": x[c * BPC : (c + 1) * BPC], "M": M, "Z": Z, "Alpha": Alpha, "rv": rv}
        for c in range(N_CORES)
    ]
    res = run_bass_kernel_spmd(nc, in_maps, list(range(N_CORES)), trace=trace)
    out = np.concatenate(
        [res.results[c]["out"] for c in range(N_CORES)], axis=0
    ).astype(np.float32)
    return out, res


def kernel(**inputs):
    out, _ = _run(inputs, trace=False)
    return out


def kernel_traced(**inputs):
    out, res = _run(inputs, trace=True)
    return out, res


# revision 11
# speedup vs baseline: 1.1684x; 1.0158x over previous
"""BinarizeConv2dSDP kernel for Trainium2 (8 NeuronCores, data-parallel over batch).

out = conv2d(sign(x), sign(M + sum_k rv[k] * Z[k]), stride 1, pad 1) * Alpha

Key simplification: the reference normalizes (M, Z) by rsqrt(M^2 + sum Z^2 / SCALE)
before forming w = rv@Z + M, but that factor is strictly positive and applied
multiplicatively to the whole expression, so sign(w) is unaffected.  The binary
weights are just sign(M + sum_k rv[k] Z[k]).

Strategy per core (8 images each):
  - every bulk HBM load is split into two halves issued on the SP (sync) and
    ACT (scalar) HWDGE rings concurrently: two issue engines keep the 16 DMA
    queues fed (~1.4x the effective startup bandwidth of a single FIFO ring),
    and the weight stream (M, Z) goes out ahead of most of image 0 so the
    weight-gen chain finishes early.
  - weight gen on DVE per ic-half: w_h = (rv0*z0_h + M_h), then += rv_k z_k_h,
    sign -> bf16 per half on ACT as soon as that half's chain is done.
    M is folded into the first scalar_tensor_tensor so no separate add.
  - 9 PE transposes (against an anti-diagonal permutation) produce the
    column-reversed, pair-interleaved fp8e4 weight layout that
    DoubleRowSwInterleave expects (plus one all-zero tap so 9 taps = 5 pairs).
  - conv: 5 fp8 DoubleRowSwInterleave matmuls per 8-row chunk (2 taps per
    matmul, K=256 effective contraction), accumulated in PSUM over a 58-wide
    zero-padded sign(x) image; the free dim spans whole padded rows (464) so
    the moving AP stays 3D, leaving 2 garbage columns per row that the
    eviction skips.
  - Alpha scaling on DVE during PSUM->SBUF eviction, emitting float16
    (exact conv integers * f32 alpha rounded to fp16: rel err ~5e-4, well
    inside the 2e-2 gate); the host upcasts to f32 after the gather.  fp16
    halves the output HBM traffic and the end-of-kernel drain tail.
  - steady-state output DMAs ride the SWDGE (GpSimd) ring so they never
    head-of-line block input loads on the HWDGE rings (the last two images'
    outputs switch to the then-idle SP ring).
"""

import numpy as np
from contextlib import ExitStack

import concourse.bass as bass
import concourse.mybir as mybir
import concourse.tile as tile
from concourse.bacc import Bacc
from concourse.bass_utils import run_bass_kernel_spmd

N_CORES = 8
B, C, H, W = 64, 128, 56, 56
BPC = B // N_CORES  # images per core
KS, K = 3, 5
PH, PW = H + 2, W + 2  # zero-padded image
CHUNK_ROWS = 8
N_CHUNKS = H // CHUNK_ROWS
FREE = CHUNK_ROWS * W  # valid output elements per chunk (448)
FREE_R = CHUNK_ROWS * PW  # matmul free dim incl. garbage cols (464 <= 512)
F32 = mybir.dt.float32
F16 = mybir.dt.float16
BF16 = mybir.dt.bfloat16
F8 = mybir.dt.float8e4

NW = C * KS * KS  # 1152 weight elements per out-channel row
NWH = NW // 2  # ic-half of the weight row (576)
HALF = (H // 2) * W  # first-half image elements (28 rows)
QTR = (H // 4) * W  # quarter image elements (14 rows)


def build_kernel(rv_vals):
    """Build the single-core Bass module (SPMD: same program on all 8 cores).

    rv_vals: the 5 rv scalars, baked as immediates into the weight-gen ops.
    """
    # Bacc (not plain Bass): its compile() pass pipeline legalizes sync waits
    # (TRN2 allows at most 1 embedded wait per engine instruction; excess waits
    # are split into InstEventSemaphore via generate_event_semaphores).
    nc = Bacc()
    x_p = nc.declare_dram_parameter("x", [BPC, C, H, W], F32, isOutput=False)
    m_p = nc.declare_dram_parameter("M", [C, C, KS, KS], F32, isOutput=False)
    z_p = nc.declare_dram_parameter("Z", [K, C, C, KS, KS], F32, isOutput=False)
    a_p = nc.declare_dram_parameter("Alpha", [C, 1, 1], F32, isOutput=False)
    rv_p = nc.declare_dram_parameter("rv", [1, K], F32, isOutput=False)
    out_p = nc.declare_dram_parameter("out", [BPC, C, H, W], F16, isOutput=True)

    with tile.TileContext(nc) as tc, ExitStack() as ctx:
        const = ctx.enter_context(tc.tile_pool(name="const", bufs=1))
        wg = ctx.enter_context(tc.tile_pool(name="wg", bufs=1))
        zpool = ctx.enter_context(tc.tile_pool(name="zpool", bufs=1))
        xin = ctx.enter_context(tc.tile_pool(name="xin", bufs=4))
        pad = ctx.enter_context(tc.tile_pool(name="pad", bufs=4))
        opool = ctx.enter_context(tc.tile_pool(name="opool", bufs=3))
        ps_t = ctx.enter_context(tc.tile_pool(name="ps_t", bufs=2, space="PSUM"))
        ps_c = ctx.enter_context(tc.tile_pool(name="ps_c", bufs=6, space="PSUM"))

        # ---- constants ----
        # Anti-diagonal permutation: transpose against it yields the transposed
        # tap with REVERSED out-channel columns, which is exactly the column
        # order DoubleRowSwInterleave's weight layout wants.
        identity = const.tile([C, C], BF16)
        nc.gpsimd.memset(identity[:], 0.0)
        nc.gpsimd.affine_select(
            out=identity[:],
            in_=identity[:],
            compare_op=mybir.AluOpType.not_equal,
            fill=1.0,
            base=-(C - 1),
            pattern=[[1, C]],
            channel_multiplier=1,
        )
        # Alpha/rv ride the SWDGE (GpSimd) ring: tiny loads, and the SP/ACT
        # HWDGE rings' first slots belong to x0/M/Z.
        alpha_sb = const.tile([C, 1], F32)
        nc.gpsimd.dma_start(alpha_sb[:], a_p[:].rearrange("c a b -> c (a b)"))
        rv_sb = const.tile([1, K], F32)
        nc.gpsimd.dma_start(rv_sb[:], rv_p[:])

        x_ap = x_p[:]
        o_ap = out_p[:]

        # ---- startup DMA schedule ----
        # Both HWDGE rings drain FIFO; each bulk tensor is split in half with
        # one half per ring so both halves stream concurrently (two issue
        # engines keep the DMA queues fed).  Wire order per ring: the first
        # quarter of image 0 (so its sign() runs early), then M and the Z
        # tensors (the weight chain is the startup critical path), then the
        # rest of image 0 quarter-by-quarter (each quarter split across both
        # rings so its sign can chase the conv chunk cadence), then images
        # 1..7.
        x_sb_0 = xin.tile([C, H * W], F32, tag="x_sb")
        xr0 = x_ap[0].rearrange("c h w -> c (h w)")
        EIG = QTR // 2  # eighth of an image (7 rows)
        # First wave on THREE rings at once: x0's first quarter rides the
        # otherwise-idle SWDGE (GpSimd) ring, M halves lead the SP/ACT rings.
        # More DMAs in flight during the queue ramp-up, and the weight stream
        # (the startup critical path) isn't delayed by any x0 bytes.
        nc.gpsimd.dma_start(x_sb_0[:, 0:EIG], xr0[:, 0:EIG])
        nc.gpsimd.dma_start(x_sb_0[:, EIG:QTR], xr0[:, EIG:QTR])

        m_sb = wg.tile([C, NW], F32)
        mr = m_p[:].rearrange("o i a b -> o (i a b)")
        nc.sync.dma_start(m_sb[:, 0:NWH], mr[:, 0:NWH])
        nc.scalar.dma_start(m_sb[:, NWH:], mr[:, NWH:])

        z_sbs = []
        NWQ = NW // 4
        for k in range(K):
            z_sbs.append(zpool.tile([C, NW], F32, name=f"z{k}", tag=f"z{k}"))
            zr = z_p[k].rearrange("o i a b -> o (i a b)")
            if k < K - 1:
                nc.sync.dma_start(z_sbs[k][:, 0:NWH], zr[:, 0:NWH])
                nc.scalar.dma_start(z_sbs[k][:, NWH:], zr[:, NWH:])
            else:
                # the last z gates the whole weight chain: quarter granularity
                # so its STT/sign tail pipelines against the stream
                nc.sync.dma_start(z_sbs[k][:, 0:NWQ], zr[:, 0:NWQ])
                nc.sync.dma_start(z_sbs[k][:, NWQ:NWH], zr[:, NWQ:NWH])
                nc.scalar.dma_start(
                    z_sbs[k][:, NWH : NWH + NWQ], zr[:, NWH : NWH + NWQ]
                )
                nc.scalar.dma_start(z_sbs[k][:, NWH + NWQ :], zr[:, NWH + NWQ :])

        for q in range(1, 4):
            lo = q * QTR
            nc.sync.dma_start(x_sb_0[:, lo : lo + EIG], xr0[:, lo : lo + EIG])
            nc.scalar.dma_start(
                x_sb_0[:, lo + EIG : lo + QTR], xr0[:, lo + EIG : lo + QTR]
            )

        # ---- weight generation: w = M + sum_k rv_k Z_k, per ic-half ----
        # Each half's chain runs on DVE as soon as its z_k half lands; M is
        # folded into the k=0 op.  Interleaving a/b ops lets DVE track the
        # two concurrent DMA streams.
        w_sb = wg.tile([C, NW], F32)
        bw_sb = wg.tile([C, NW], BF16)
        halves = (slice(0, NWH), slice(NWH, NW))
        for s in halves:
            nc.vector.scalar_tensor_tensor(
                w_sb[:, s],
                z_sbs[0][:, s],
                float(rv_vals[0]),
                m_sb[:, s],
                mybir.AluOpType.mult,
                mybir.AluOpType.add,
            )
        for k in range(1, K - 1):
            for s in halves:
                nc.vector.scalar_tensor_tensor(
                    w_sb[:, s],
                    z_sbs[k][:, s],
                    float(rv_vals[k]),
                    w_sb[:, s],
                    mybir.AluOpType.mult,
                    mybir.AluOpType.add,
                )
        # z4 arrives as four quarters, (0:288 | 576:864) first (one per ring):
        # emit its STT+sign pieces in arrival order so the tail pipelines
        # against the last bytes of the stream.
        for s in (
            slice(0, NWQ),
            slice(NWH, NWH + NWQ),
            slice(NWQ, NWH),
            slice(NWH + NWQ, NW),
        ):
            nc.vector.scalar_tensor_tensor(
                w_sb[:, s],
                z_sbs[K - 1][:, s],
                float(rv_vals[K - 1]),
                w_sb[:, s],
                mybir.AluOpType.mult,
                mybir.AluOpType.add,
            )
            nc.scalar.sign(bw_sb[:, s], w_sb[:, s])

        # Transpose each tap's [oc, ic] into [ic, oc-reversed] (via the
        # anti-diagonal permutation), then interleave tap pairs column-wise as
        # fp8e4 (+-1 exact): [A127 B127 A126 B126 ... A0 B0] per partition --
        # the DoubleRowSwInterleave weight layout.  Pre-interleaving makes
        # LDWEIGHTS a single 128-column pass instead of DoubleRow's 256-column
        # reload.  Tap 9 (pair 4, slot B) stays all-zero.
        wt = const.tile([C, 5, 2 * C], F8)
        nc.gpsimd.memset(wt[:, 4, :], 0.0)
        bw_r = bw_sb[:].rearrange("o (i j) -> o i j", j=KS * KS)

        def wt_step(j):
            """Transpose tap j and copy it into its interleaved wt slot."""
            tp = ps_t.tile([C, C], BF16)
            nc.tensor.transpose(tp[:], bw_r[:, :, j], identity[:])
            pair, slot = divmod(j, 2)
            wt_h = wt[:].tensor
            dst = bass.AP(wt_h, pair * 2 * C + slot, [[5 * 2 * C, C], [2, C]])
            nc.vector.tensor_copy(dst, tp[:])

        # rv reaches the kernel as baked immediates; touch the tensor so the
        # bound input isn't dead.
        nc.vector.tensor_copy(w_sb[0:1, 0:K], rv_sb[0:1, :])

        def tap_off(r0, j):
            # flat offset of (out-row r0, tap j)'s top-left read in the padded image
            if j == KS * KS:  # zero tap: alias tap 8's window (weights are 0)
                j = KS * KS - 1
            return (r0 + j // KS) * PW + (j % KS)

        def load_sign(i):
            """Image load + binarize into a fresh zero-bordered pad tile."""
            if i == 0:
                x_sb = x_sb_0
            else:
                x_sb = xin.tile([C, H * W], F32, tag="x_sb")
                xr = x_ap[i].rearrange("c h w -> c (h w)")
                nc.sync.dma_start(x_sb[:, 0:HALF], xr[:, 0:HALF])
                nc.scalar.dma_start(x_sb[:, HALF:], xr[:, HALF:])
            ba = pad.tile([C, PH * PW + 2], F8, tag="ba")
            ba_r = ba[:, 0 : PH * PW].rearrange("c (h w) -> c h w", w=PW)
            # Zero only the pad border (sign() fills the interior); GpSimd is
            # idle here and keeps these off the DVE eviction path.
            nc.gpsimd.memset(ba[:, 0:PW], 0.0)
            nc.gpsimd.memset(ba[:, (PH - 1) * PW : PH * PW + 2], 0.0)
            nc.gpsimd.memset(ba_r[:, 1 : H + 1, 0:1], 0.0)
            nc.gpsimd.memset(ba_r[:, 1 : H + 1, W + 1 : PW], 0.0)
            x_r = x_sb[:].rearrange("c (h w) -> c h w", w=W)
            if i == 0:
                # eighth-granularity signs: each piece runs as soon as its DMA
                # lands, and a ~0.5us piece never blocks the (critical) weight
                # signs on the ACT queue for long
                for q in range(8):
                    r0, r1 = q * (H // 8), (q + 1) * (H // 8)
                    nc.scalar.sign(
                        ba_r[:, r0 + 1 : r1 + 1, 1 : W + 1], x_r[:, r0:r1]
                    )
            else:
                nc.scalar.sign(ba_r[:, 1 : H // 2 + 1, 1 : W + 1], x_r[:, : H // 2])
                nc.scalar.sign(ba_r[:, H // 2 + 1 : H + 1, 1 : W + 1], x_r[:, H // 2 :])
            return ba

        def conv_store(i, ba):
            """9-tap binary conv via 5 DoubleRow matmuls per chunk + eviction.

            For image 0 the 9 weight transposes are interleaved with chunk 0's
            matmuls (pair p's matmul is emitted right after its two taps land
            in wt), so the conv starts as soon as the first weight pair is
            ready instead of after all nine transposes.
            """
            o_sb = opool.tile([C, H * W], F16, tag="o_sb")
            for ch in range(N_CHUNKS):
                pt = ps_c.tile([C, FREE_R], F32, tag="pt")
                r0 = ch * CHUNK_ROWS
                for p in range(5):
                    if i == 0 and ch == 0:
                        for j in (2 * p, 2 * p + 1):
                            if j < KS * KS:
                                wt_step(j)
                    o0 = tap_off(r0, 2 * p)
                    o1 = tap_off(r0, 2 * p + 1)
                    rhs = bass.AP(
                        ba[:].tensor,
                        o0,
                        [[PH * PW + 2, C], [o1 - o0, 2], [1, FREE_R]],
                    )
                    nc.tensor.matmul(
                        pt[:],
                        wt[:, p, :],
                        rhs,
                        start=(p == 0),
                        stop=(p == 4),
                        perf_mode=mybir.MatmulPerfMode.DoubleRowSwInterleave,
                    )
                # PSUM -> SBUF eviction with per-channel Alpha scale on DVE,
                # casting to fp16 and skipping the 2 garbage columns per row.
                nc.vector.tensor_scalar_mul(
                    o_sb[:, ch * FREE : (ch + 1) * FREE].rearrange(
                        "c (a b) -> c a b", b=W
                    ),
                    pt[:].rearrange("c (a b) -> c a b", b=PW)[:, :, 0:W],
                    alpha_sb[:, 0:1],
                )
                # Output DMAs ride the SWDGE (GpSimd) ring: an output DMA
                # waiting on evictions would head-of-line block later input
                # loads on the HWDGE rings.  Half-image granularity shrinks
                # the end-of-kernel tail.  Images >= 6 finish after every
                # input load has drained, so their outputs ride the idle SP
                # HWDGE ring (cheaper issue, no head-of-line risk anymore).
                out_dma = nc.sync.dma_start if i >= 6 else nc.gpsimd.dma_start
                last_img = i == BPC - 1
                if last_img:
                    # per-chunk pieces all the way through the final image:
                    # the drain tail is one eviction + one small DMA.  The
                    # final piece rides the (idle) ACT ring so it doesn't
                    # queue behind earlier pieces on the SP ring.
                    (nc.scalar.dma_start if ch == N_CHUNKS - 1 else out_dma)(
                        o_ap[i].rearrange("c h w -> c (h w)")[
                            :, ch * FREE : (ch + 1) * FREE
                        ],
                        o_sb[:, ch * FREE : (ch + 1) * FREE],
                    )
                elif ch == 3:
                    out_dma(
                        o_ap[i].rearrange("c h w -> c (h w)")[:, 0 : 4 * FREE],
                        o_sb[:, 0 : 4 * FREE],
                    )
                elif ch == N_CHUNKS - 1:
                    out_dma(
                        o_ap[i].rearrange("c h w -> c (h w)")[:, 4 * FREE :],
                        o_sb[:, 4 * FREE :],
                    )

        # Software-pipelined: image i+1's load/sign issues before image i's
        # conv+store so ScalarE signs (and input DMAs) always run ahead.
        prev_ba = None
        for i in range(BPC):
            ba = load_sign(i)
            if prev_ba is not None:
                conv_store(i - 1, prev_ba)
            prev_ba = ba
        conv_store(BPC - 1, prev_ba)

    nc.finalize()
    return nc


_CACHE = {}


def _get_nc(rv):
    key = rv.tobytes()
    if key not in _CACHE:
        _CACHE[key] = build_kernel(np.asarray(rv, np.float32).reshape(-1))
    return _CACHE[key]


def _run(inputs, trace=False):
    x = np.ascontiguousarray(np.asarray(inputs["x"], np.float32))
    M = np.ascontiguousarray(np.asarray(inputs["M"], np.float32))
    Z = np.ascontiguousarray(np.asarray(inputs["Z"], np.float32))
    Alpha = np.ascontiguousarray(np.asarray(inputs["Alpha"], np.float32))
    rv = np.ascontiguousarray(np.asarray(inputs["rv"], np.float32))
    nc = _get_nc(rv)
    in_maps = [
        {"x": x[c * BPC : (c + 1) * BPC], "M": M, "Z": Z, "Alpha": Alpha, "rv": rv}
        for c in range(N_CORES)
    ]
    res = run_bass_kernel_spmd(nc, in_maps, list(range(N_CORES)), trace=trace)
    out = np.concatenate(
        [res.results[c]["out"] for c in range(N_CORES)], axis=0
    ).astype(np.float32)
    return out, res


def kernel(**inputs):
    out, _ = _run(inputs, trace=False)
    return out


def kernel_traced(**inputs):
    out, res = _run(inputs, trace=True)
    return out, res
